# revision 7
# baseline (speedup 1.0000x reference)
"""TRN2 Bass kernel for CompressedLinearLayer: out = x @ (A @ B.T).T + bias.

Computed low-rank: t = x @ B  (rank 512), out = t @ A.T  (+ bias on host).
Sharding: data-parallel over the 8192 rows of x (1024 rows per core);
B, A.T replicated. No collectives.

Schedule (per core): all of stage 1 first (256 MMs, contracting d_in=4096
into 8 resident PSUM banks for all 1024 rows), then all of stage 2
(256+ MMs over 32 units of 128 rows x 1024 d_out). This keeps the HBM
read demand flat (~220 GB/s) under the 358 GB/s/core limit so the PE
never starves. B and x ride one HWDGE ring (sync) interleaved in exact
consumption order; A.T rides the scalar ring, ordering-delayed behind
the mid-stage-1 x stream; output stores alternate rings.

All device DMAs are fully contiguous (inputs pre-tiled on host into the
exact SBUF layouts). Output is written bf16 (halves store traffic, ~0.2%
extra rounding) and upconverted + bias-added on host. Dummy warm-up
matmuls run while the first x/B pieces stream in so the PE's HAM
clock-gate is released right when the real stream begins. The final
stage-2 unit is split into 4 fine pieces to shrink the exposed
cast+store+HBM-receipt tail.
"""
import numpy as np
import ml_dtypes

import concourse.bacc as bacc
import concourse.mybir as mybir
import concourse.tile as tile
from concourse.tile import add_dep_helper
from concourse.bass_utils import run_bass_kernel_spmd

N_CORES = 8
BATCH, SEQ = 4, 2048
D_IN, D_OUT, RANK = 4096, 4096, 512
ROWS_TOTAL = BATCH * SEQ           # 8192
ROWS = ROWS_TOTAL // N_CORES       # 1024 rows per core

F32 = mybir.dt.float32
BF16 = mybir.dt.bfloat16

KC = D_IN // 128     # 32 contraction chunks, stage 1
KSUB = 4             # k-chunks per DMA group (1MB x, 0.5MB B transfers)
KB = KC // KSUB      # 8 groups
RC = RANK // 128     # 4 rank chunks
NRH = ROWS // 512    # 2 row halves (psum moving-dim limit is 512 fp32)
WCOLS = 1024         # stage-2 d_out window per unit
NW = D_OUT // WCOLS  # 4 A.T windows
RC2 = ROWS // 128    # 8 stage-2 row chunks
N_WARMUP = 16        # dummy MMs (N=256, ~213ns cold) ~= 3.4us HAM window

_compiled = {}


def _build():
    nc = bacc.Bacc("TRN2", target_bir_lowering=False, debug=False)

    # host-pretiled: xt[g*128+p, ks, m] = x_shard[m, (g*KSUB+ks)*128+p]
    xt_d = nc.declare_dram_parameter("xt", [KB * 128, KSUB, ROWS], BF16, isOutput=False)
    # b[g*128+p, ks, r] = B[(g*KSUB+ks)*128+p, r]
    b_d = nc.declare_dram_parameter("b", [KB * 128, KSUB, RANK], BF16, isOutput=False)
    # atw[w*128+p, k, c] = A.T[k*128+p, w*WCOLS+c] = A[w*WCOLS+c, k*128+p]
    atw_d = nc.declare_dram_parameter("atw", [NW * 128, RC, WCOLS], BF16, isOutput=False)
    out_d = nc.declare_dram_parameter("out", [ROWS, D_OUT], BF16, isOutput=True)

    with tile.TileContext(nc) as tc:
        with (
            tc.tile_pool(name="wb", bufs=1) as wb,
            tc.tile_pool(name="xp", bufs=4) as xp,
            tc.tile_pool(name="op", bufs=4) as op,
            tc.tile_pool(name="psp", bufs=8, space="PSUM") as psp,
        ):
            # --- PE warm-up: garbage MMs with no DMA dependency ---
            wu_a = wb.tile([128, 128], BF16, tag="wu_a")
            wu_b = wb.tile([128, 256], BF16, tag="wu_b")
            nc.vector.memset(wu_a[:], 0.0)
            nc.vector.memset(wu_b[:], 0.0)
            wu_ps = psp.tile([128, 512], F32, tag="ps", name="wu_ps")
            for i in range(N_WARMUP):
                nc.tensor.matmul(
                    wu_ps[:, 0:256], wu_a[:], wu_b[:], start=True, stop=True
                )

            # B resident: 8 tiles [128, KSUB, 512] bf16 (0.5MB each)
            b_sb = [
                wb.tile([128, KSUB, RANK], BF16, tag=f"b{g}", name=f"b{g}")
                for g in range(KB)
            ]
            # A.T windows: 4 tiles [128, RC, 1024] bf16 (1MB each)
            at_sb = [
                wb.tile([128, RC, WCOLS], BF16, tag=f"at{w}", name=f"at{w}")
                for w in range(NW)
            ]
            # t resident: [rh][mc] -> [128 rank, 512 rows] bf16
            tT = [
                [
                    wb.tile([128, 512], BF16, tag=f"tT{rh}_{mc}", name=f"tT{rh}_{mc}")
                    for mc in range(RC)
                ]
                for rh in range(NRH)
            ]

            # stage-1 PSUM: 8 banks live for the whole contraction,
            # allocation order == evacuation order (rh-major) so stage-2
            # allocations rotate into the first-freed banks.
            ps1 = [
                [
                    psp.tile([128, 512], F32, tag="ps", name=f"ps1_{rh}_{mc}")
                    for mc in range(RC)
                ]
                for rh in range(NRH)
            ]

            # --- stage 1: t[r, m] = sum_k B[k, r] * x[m, k] ---
            # B and x interleaved on one ring in exact consumption order.
            x_group_dma = {}
            for g in range(KB):
                xg = xp.tile([128, KSUB, ROWS], BF16, tag="xk", name=f"x{g}")
                if g == 0:
                    nc.sync.dma_start(b_sb[0][:, 0:1, :], b_d[0:128, 0:1, :])
                    nc.sync.dma_start(xg[:, 0:1, 0:512], xt_d[0:128, 0:1, 0:512])
                    nc.sync.dma_start(xg[:, 0:1, 512:1024], xt_d[0:128, 0:1, 512:1024])
                    nc.sync.dma_start(b_sb[0][:, 1:KSUB, :], b_d[0:128, 1:KSUB, :])
                    for ks in range(1, KSUB):
                        d = nc.sync.dma_start(
                            xg[:, ks:ks + 1, :], xt_d[0:128, ks:ks + 1, :]
                        )
                    x_group_dma[0] = d
                else:
                    nc.sync.dma_start(
                        b_sb[g][:], b_d[g * 128:(g + 1) * 128, :, :]
                    )
                    x_group_dma[g] = nc.sync.dma_start(
                        xg[:], xt_d[g * 128:(g + 1) * 128, :, :]
                    )
                if g == 0:
                    # rh outer so the first 4 MMs only need the rh0 piece
                    for ks in range(KSUB):
                        for rh in range(NRH):
                            for mc in range(RC):
                                nc.tensor.matmul(
                                    ps1[rh][mc][:],
                                    b_sb[g][:, ks, mc * 128:(mc + 1) * 128],
                                    xg[:, ks, rh * 512:(rh + 1) * 512],
                                    start=(ks == 0),
                                    stop=False,
                                )
                elif g < KB - 1:
                    for ks in range(KSUB):
                        for mc in range(RC):
                            for rh in range(NRH):
                                nc.tensor.matmul(
                                    ps1[rh][mc][:],
                                    b_sb[g][:, ks, mc * 128:(mc + 1) * 128],
                                    xg[:, ks, rh * 512:(rh + 1) * 512],
                                    start=False,
                                    stop=False,
                                )
                else:
                    # last group: finish each psum in rh-major order and
                    # evacuate on the DVE while the PE continues
                    for rh in range(NRH):
                        for mc in range(RC):
                            for ks in range(KSUB):
                                nc.tensor.matmul(
                                    ps1[rh][mc][:],
                                    b_sb[g][:, ks, mc * 128:(mc + 1) * 128],
                                    xg[:, ks, rh * 512:(rh + 1) * 512],
                                    start=False,
                                    stop=(ks == KSUB - 1),
                                )
                            nc.vector.tensor_copy(tT[rh][mc][:], ps1[rh][mc][:])

            # A.T prefetch on the scalar ring, ordering-delayed behind the
            # mid-stage-1 x stream so it never contends with the prologue.
            for w in range(NW):
                at_dma = nc.scalar.dma_start(
                    at_sb[w][:], atw_d[w * 128:(w + 1) * 128, :, :]
                )
                add_dep_helper(
                    at_dma.ins,
                    x_group_dma[min(2 + w, KB - 1)].ins,
                    sync=True,
                    reason="delay A.T load behind stage-1 x stream",
                )

            # --- stage 2: out[m, d] = sum_r t[r, m] * A.T[r, d] ---
            for w in range(NW):
                for rc2 in range(RC2):
                    rh, r0 = rc2 // 4, (rc2 % 4) * 128
                    last = (w == NW - 1) and (rc2 == RC2 - 1)
                    store_eng = nc.sync if (w * RC2 + rc2) % 2 == 0 else nc.scalar
                    ot = op.tile([128, WCOLS], BF16, tag="ot", name=f"ot{w}_{rc2}")
                    if not last:
                        for dc in range(2):
                            ps2 = psp.tile(
                                [128, 512], F32, tag="ps", name=f"ps2_{w}_{rc2}_{dc}"
                            )
                            for k in range(RC):
                                nc.tensor.matmul(
                                    ps2[:],
                                    tT[rh][k][:, r0:r0 + 128],
                                    at_sb[w][:, k, dc * 512:(dc + 1) * 512],
                                    start=(k == 0),
                                    stop=(k == RC - 1),
                                )
                            nc.vector.tensor_copy(
                                ot[:, dc * 512:(dc + 1) * 512], ps2[:]
                            )
                        store_eng.dma_start(
                            out_d[rc2 * 128:(rc2 + 1) * 128, w * WCOLS:(w + 1) * WCOLS],
                            ot[:],
                        )
                    else:
                        # final unit: fine-grained so the exposed tail
                        # (cast + store + HBM receipt) is minimal
                        for dcq in range(4):
                            ps2 = psp.tile(
                                [128, 512], F32, tag="ps", name=f"ps2_last_{dcq}"
                            )
                            for k in range(RC):
                                nc.tensor.matmul(
                                    ps2[:, 0:256],
                                    tT[rh][k][:, r0:r0 + 128],
                                    at_sb[w][:, k, dcq * 256:(dcq + 1) * 256],
                                    start=(k == 0),
                                    stop=(k == RC - 1),
                                )
                            nc.vector.tensor_copy(
                                ot[:, dcq * 256:(dcq + 1) * 256], ps2[:, 0:256]
                            )
                            eng = nc.sync if dcq % 2 == 0 else nc.scalar
                            eng.dma_start(
                                out_d[
                                    rc2 * 128:(rc2 + 1) * 128,
                                    w * WCOLS + dcq * 256:w * WCOLS + (dcq + 1) * 256,
                                ],
                                ot[:, dcq * 256:(dcq + 1) * 256],
                            )

    nc.compile()
    return nc


def _get_nc():
    if "nc" not in _compiled:
        _compiled["nc"] = _build()
    return _compiled["nc"]


def _prep_shared(A, B):
    # b[g][p][ks][r] = B[(g*KSUB+ks)*128+p, r]
    b_t = np.ascontiguousarray(
        B.reshape(KB, KSUB, 128, RANK).transpose(0, 2, 1, 3)
    ).astype(ml_dtypes.bfloat16).reshape(KB * 128, KSUB, RANK)
    # atw[w][p][k][c] = A.T[k*128+p, w*WCOLS+c]
    AT = np.ascontiguousarray(A.T)  # [RANK, D_OUT]
    atw = np.ascontiguousarray(
        AT.reshape(RC, 128, NW, WCOLS).transpose(2, 1, 0, 3)
    ).astype(ml_dtypes.bfloat16).reshape(NW * 128, RC, WCOLS)
    return b_t, atw


def run(inputs, trace=False, trace_kwargs=None):
    """Shard, execute on 8 cores, gather. Returns (output, BassKernelResults)."""
    x = np.asarray(inputs["x"], dtype=np.float32)
    A = np.asarray(inputs["A"], dtype=np.float32)
    B = np.asarray(inputs["B"], dtype=np.float32)
    bias = np.asarray(inputs["bias"], dtype=np.float32)

    x_flat = x.reshape(ROWS_TOTAL, D_IN)
    b_t, atw = _prep_shared(A, B)
    in_maps = []
    for i in range(N_CORES):
        xs = x_flat[i * ROWS:(i + 1) * ROWS]  # [ROWS, D_IN]
        # xt[g][p][ks][m] = xs[m, (g*KSUB+ks)*128+p]
        xt = np.ascontiguousarray(
            xs.T.reshape(KB, KSUB, 128, ROWS).transpose(0, 2, 1, 3)
        ).astype(ml_dtypes.bfloat16).reshape(KB * 128, KSUB, ROWS)
        in_maps.append({"xt": xt, "b": b_t, "atw": atw})

    nc = _get_nc()
    kwargs = {}
    if trace:
        kwargs["trace"] = True
        kwargs["trace_kwargs"] = trace_kwargs or {}
    res = None
    for attempt in range(3):
        try:
            res = run_bass_kernel_spmd(
                nc, in_maps, core_ids=list(range(N_CORES)), **kwargs
            )
        except Exception:
            # transient device/runtime hiccup; retry
            if attempt == 2:
                raise
            continue
        out = np.concatenate(
            [np.asarray(res.results[i]["out"]) for i in range(N_CORES)], axis=0
        )
        out = out.astype(np.float32) + bias[None, :]
        if np.isfinite(out).all():
            return out.reshape(BATCH, SEQ, D_OUT), res
    return out.reshape(BATCH, SEQ, D_OUT), res


def kernel(**inputs) -> np.ndarray:
    out, _ = run(inputs)
    return out


# revision 9
# speedup vs baseline: 1.0142x; 1.0142x over previous
"""TRN2 Bass kernel for CompressedLinearLayer: out = x @ (A @ B.T).T + bias.

Computed low-rank: t = x @ B  (rank 512), out = t @ A.T  (+ bias on host).
Sharding: data-parallel over the 8192 rows of x (1024 rows per core);
B, A.T replicated. No collectives.

Schedule (per core): all of stage 1 first (256 MMs, contracting d_in=4096
into 8 resident PSUM banks for all 1024 rows), then all of stage 2
(256+ MMs over 32 units of 128 rows x 1024 d_out). This keeps the HBM
read demand flat (~220 GB/s) under the 358 GB/s/core limit so the PE
never starves. B and x ride one HWDGE ring (sync) interleaved in exact
consumption order; A.T rides the scalar ring, ordering-delayed behind
the mid-stage-1 x stream; output stores alternate rings.

All device DMAs are fully contiguous (inputs pre-tiled on host into the
exact SBUF layouts). Output is written bf16 (halves store traffic, ~0.2%
extra rounding) and upconverted + bias-added on host. Dummy warm-up
matmuls run while the first x/B pieces stream in so the PE's HAM
clock-gate is released right when the real stream begins. The final
stage-2 unit is split into 4 fine pieces to shrink the exposed
cast+store+HBM-receipt tail.
"""
import numpy as np
import ml_dtypes

import concourse.bacc as bacc
import concourse.mybir as mybir
import concourse.tile as tile
from concourse.tile import add_dep_helper
from concourse.bass_utils import run_bass_kernel_spmd

N_CORES = 8
BATCH, SEQ = 4, 2048
D_IN, D_OUT, RANK = 4096, 4096, 512
ROWS_TOTAL = BATCH * SEQ           # 8192
ROWS = ROWS_TOTAL // N_CORES       # 1024 rows per core

F32 = mybir.dt.float32
BF16 = mybir.dt.bfloat16

KC = D_IN // 128     # 32 contraction chunks, stage 1
KSUB = 4             # k-chunks per DMA group (1MB x, 0.5MB B transfers)
KB = KC // KSUB      # 8 groups
RC = RANK // 128     # 4 rank chunks
NRH = ROWS // 512    # 2 row halves (psum moving-dim limit is 512 fp32)
WCOLS = 1024         # stage-2 d_out window per unit
NW = D_OUT // WCOLS  # 4 A.T windows
RC2 = ROWS // 128    # 8 stage-2 row chunks
N_WARMUP = 16        # dummy MMs (N=256, ~213ns cold) ~= 3.4us HAM window

_compiled = {}


def _build():
    nc = bacc.Bacc("TRN2", target_bir_lowering=False, debug=False)

    # host-pretiled: xt[g*128+p, ks, m] = x_shard[m, (g*KSUB+ks)*128+p]
    xt_d = nc.declare_dram_parameter("xt", [KB * 128, KSUB, ROWS], BF16, isOutput=False)
    # b[g*128+p, ks, r] = B[(g*KSUB+ks)*128+p, r]
    b_d = nc.declare_dram_parameter("b", [KB * 128, KSUB, RANK], BF16, isOutput=False)
    # atw[w*128+p, k, c] = A.T[k*128+p, w*WCOLS+c] = A[w*WCOLS+c, k*128+p]
    atw_d = nc.declare_dram_parameter("atw", [NW * 128, RC, WCOLS], BF16, isOutput=False)
    out_d = nc.declare_dram_parameter("out", [ROWS, D_OUT], BF16, isOutput=True)

    with tile.TileContext(nc) as tc:
        with (
            tc.tile_pool(name="wb", bufs=1) as wb,
            tc.tile_pool(name="xp", bufs=4) as xp,
            tc.tile_pool(name="op", bufs=4) as op,
            tc.tile_pool(name="psp", bufs=8, space="PSUM") as psp,
        ):
            # --- PE warm-up: garbage MMs with no DMA dependency ---
            wu_a = wb.tile([128, 128], BF16, tag="wu_a")
            wu_b = wb.tile([128, 256], BF16, tag="wu_b")
            nc.vector.memset(wu_a[:], 0.0)
            nc.vector.memset(wu_b[:], 0.0)
            wu_ps = psp.tile([128, 512], F32, tag="ps", name="wu_ps")
            for i in range(N_WARMUP):
                nc.tensor.matmul(
                    wu_ps[:, 0:256], wu_a[:], wu_b[:], start=True, stop=True
                )

            # B resident: 8 tiles [128, KSUB, 512] bf16 (0.5MB each)
            b_sb = [
                wb.tile([128, KSUB, RANK], BF16, tag=f"b{g}", name=f"b{g}")
                for g in range(KB)
            ]
            # A.T windows: 4 tiles [128, RC, 1024] bf16 (1MB each)
            at_sb = [
                wb.tile([128, RC, WCOLS], BF16, tag=f"at{w}", name=f"at{w}")
                for w in range(NW)
            ]
            # t resident: [rh][mc] -> [128 rank, 512 rows] bf16
            tT = [
                [
                    wb.tile([128, 512], BF16, tag=f"tT{rh}_{mc}", name=f"tT{rh}_{mc}")
                    for mc in range(RC)
                ]
                for rh in range(NRH)
            ]

            # stage-1 PSUM: 8 banks live for the whole contraction,
            # allocation order == evacuation order (rh-major) so stage-2
            # allocations rotate into the first-freed banks.
            ps1 = [
                [
                    psp.tile([128, 512], F32, tag="ps", name=f"ps1_{rh}_{mc}")
                    for mc in range(RC)
                ]
                for rh in range(NRH)
            ]

            # --- stage 1: t[r, m] = sum_k B[k, r] * x[m, k] ---
            # B and x interleaved on one ring in exact consumption order.
            x_group_dma = {}
            for g in range(KB):
                xg = xp.tile([128, KSUB, ROWS], BF16, tag="xk", name=f"x{g}")
                if g == 0:
                    nc.scalar.dma_start(b_sb[0][:, 0:1, :], b_d[0:128, 0:1, :])
                    nc.sync.dma_start(xg[:, 0:1, 0:512], xt_d[0:128, 0:1, 0:512])
                    nc.sync.dma_start(xg[:, 0:1, 512:1024], xt_d[0:128, 0:1, 512:1024])
                    nc.scalar.dma_start(b_sb[0][:, 1:KSUB, :], b_d[0:128, 1:KSUB, :])
                    for ks in range(1, KSUB):
                        d = nc.sync.dma_start(
                            xg[:, ks:ks + 1, :], xt_d[0:128, ks:ks + 1, :]
                        )
                    x_group_dma[0] = d
                else:
                    nc.scalar.dma_start(
                        b_sb[g][:], b_d[g * 128:(g + 1) * 128, :, :]
                    )
                    x_group_dma[g] = nc.sync.dma_start(
                        xg[:], xt_d[g * 128:(g + 1) * 128, :, :]
                    )
                if g == 0:
                    # rh outer so the first 4 MMs only need the rh0 piece
                    for ks in range(KSUB):
                        for rh in range(NRH):
                            for mc in range(RC):
                                nc.tensor.matmul(
                                    ps1[rh][mc][:],
                                    b_sb[g][:, ks, mc * 128:(mc + 1) * 128],
                                    xg[:, ks, rh * 512:(rh + 1) * 512],
                                    start=(ks == 0),
                                    stop=False,
                                )
                elif g < KB - 1:
                    for ks in range(KSUB):
                        for mc in range(RC):
                            for rh in range(NRH):
                                nc.tensor.matmul(
                                    ps1[rh][mc][:],
                                    b_sb[g][:, ks, mc * 128:(mc + 1) * 128],
                                    xg[:, ks, rh * 512:(rh + 1) * 512],
                                    start=False,
                                    stop=False,
                                )
                else:
                    # last group: finish each psum in rh-major order and
                    # evacuate on the DVE while the PE continues
                    for rh in range(NRH):
                        for mc in range(RC):
                            for ks in range(KSUB):
                                nc.tensor.matmul(
                                    ps1[rh][mc][:],
                                    b_sb[g][:, ks, mc * 128:(mc + 1) * 128],
                                    xg[:, ks, rh * 512:(rh + 1) * 512],
                                    start=False,
                                    stop=(ks == KSUB - 1),
                                )
                            nc.vector.tensor_copy(tT[rh][mc][:], ps1[rh][mc][:])

            # A.T prefetch on the scalar ring, ordering-delayed behind the
            # mid-stage-1 x stream so it never contends with the prologue.
            for w in range(NW):
                at_dma = nc.scalar.dma_start(
                    at_sb[w][:], atw_d[w * 128:(w + 1) * 128, :, :]
                )
                add_dep_helper(
                    at_dma.ins,
                    x_group_dma[min(2 + w, KB - 1)].ins,
                    sync=True,
                    reason="delay A.T load behind stage-1 x stream",
                )

            # --- stage 2: out[m, d] = sum_r t[r, m] * A.T[r, d] ---
            for w in range(NW):
                for rc2 in range(RC2):
                    rh, r0 = rc2 // 4, (rc2 % 4) * 128
                    last = (w == NW - 1) and (rc2 == RC2 - 1)
                    store_eng = nc.sync if (w * RC2 + rc2) % 2 == 0 else nc.scalar
                    ot = op.tile([128, WCOLS], BF16, tag="ot", name=f"ot{w}_{rc2}")
                    if not last:
                        for dc in range(2):
                            ps2 = psp.tile(
                                [128, 512], F32, tag="ps", name=f"ps2_{w}_{rc2}_{dc}"
                            )
                            for k in range(RC):
                                nc.tensor.matmul(
                                    ps2[:],
                                    tT[rh][k][:, r0:r0 + 128],
                                    at_sb[w][:, k, dc * 512:(dc + 1) * 512],
                                    start=(k == 0),
                                    stop=(k == RC - 1),
                                )
                            nc.vector.tensor_copy(
                                ot[:, dc * 512:(dc + 1) * 512], ps2[:]
                            )
                        store_eng.dma_start(
                            out_d[rc2 * 128:(rc2 + 1) * 128, w * WCOLS:(w + 1) * WCOLS],
                            ot[:],
                        )
                    else:
                        # final unit tapered (512/256/128/128 cols) so the
                        # exposed tail (cast + store + HBM receipt) is minimal
                        pieces = [(0, 512, nc.scalar), (512, 256, nc.sync),
                                  (768, 128, nc.scalar), (896, 128, nc.sync)]
                        for pi, (c0, cn, eng) in enumerate(pieces):
                            ps2 = psp.tile(
                                [128, 512], F32, tag="ps", name=f"ps2_last_{pi}"
                            )
                            for k in range(RC):
                                nc.tensor.matmul(
                                    ps2[:, 0:cn],
                                    tT[rh][k][:, r0:r0 + 128],
                                    at_sb[w][:, k, c0:c0 + cn],
                                    start=(k == 0),
                                    stop=(k == RC - 1),
                                )
                            nc.vector.tensor_copy(
                                ot[:, c0:c0 + cn], ps2[:, 0:cn]
                            )
                            eng.dma_start(
                                out_d[
                                    rc2 * 128:(rc2 + 1) * 128,
                                    w * WCOLS + c0:w * WCOLS + c0 + cn,
                                ],
                                ot[:, c0:c0 + cn],
                            )

    nc.compile()
    return nc


def _get_nc():
    if "nc" not in _compiled:
        _compiled["nc"] = _build()
    return _compiled["nc"]


def _prep_shared(A, B):
    # b[g][p][ks][r] = B[(g*KSUB+ks)*128+p, r]
    b_t = np.ascontiguousarray(
        B.reshape(KB, KSUB, 128, RANK).transpose(0, 2, 1, 3)
    ).astype(ml_dtypes.bfloat16).reshape(KB * 128, KSUB, RANK)
    # atw[w][p][k][c] = A.T[k*128+p, w*WCOLS+c]
    AT = np.ascontiguousarray(A.T)  # [RANK, D_OUT]
    atw = np.ascontiguousarray(
        AT.reshape(RC, 128, NW, WCOLS).transpose(2, 1, 0, 3)
    ).astype(ml_dtypes.bfloat16).reshape(NW * 128, RC, WCOLS)
    return b_t, atw


def run(inputs, trace=False, trace_kwargs=None):
    """Shard, execute on 8 cores, gather. Returns (output, BassKernelResults)."""
    x = np.asarray(inputs["x"], dtype=np.float32)
    A = np.asarray(inputs["A"], dtype=np.float32)
    B = np.asarray(inputs["B"], dtype=np.float32)
    bias = np.asarray(inputs["bias"], dtype=np.float32)

    x_flat = x.reshape(ROWS_TOTAL, D_IN)
    b_t, atw = _prep_shared(A, B)
    in_maps = []
    for i in range(N_CORES):
        xs = x_flat[i * ROWS:(i + 1) * ROWS]  # [ROWS, D_IN]
        # xt[g][p][ks][m] = xs[m, (g*KSUB+ks)*128+p]
        xt = np.ascontiguousarray(
            xs.T.reshape(KB, KSUB, 128, ROWS).transpose(0, 2, 1, 3)
        ).astype(ml_dtypes.bfloat16).reshape(KB * 128, KSUB, ROWS)
        in_maps.append({"xt": xt, "b": b_t, "atw": atw})

    nc = _get_nc()
    kwargs = {}
    if trace:
        kwargs["trace"] = True
        kwargs["trace_kwargs"] = trace_kwargs or {}
    res = None
    for attempt in range(3):
        try:
            res = run_bass_kernel_spmd(
                nc, in_maps, core_ids=list(range(N_CORES)), **kwargs
            )
        except Exception:
            # transient device/runtime hiccup; retry
            if attempt == 2:
                raise
            continue
        out = np.concatenate(
            [np.asarray(res.results[i]["out"]) for i in range(N_CORES)], axis=0
        )
        out = out.astype(np.float32) + bias[None, :]
        if np.isfinite(out).all():
            return out.reshape(BATCH, SEQ, D_OUT), res
    return out.reshape(BATCH, SEQ, D_OUT), res


def kernel(**inputs) -> np.ndarray:
    out, _ = run(inputs)
    return out
